# revision 1
# baseline (speedup 1.0000x reference)
"""Trainium2 Bass kernel for nn_Attn_47072841564500 (sparse_attention).

Reference computation:
    proj   = einsum('sbn,mn->sbm', encoder_outputs, W) + b     # [S, B, N]
    scores = einsum('bn,sbn->bs', hidden[0], proj)             # [B, S]
    attn   = softmax(scores, axis=1)[:, None, :]               # [B, 1, S]

Key algebraic reduction: scores[b,s] = sum_n enc[s,b,n] * u[b,n] + hidden[b]@bias
with u = hidden[0] @ W.  The bias term is constant per softmax row, and softmax
is shift-invariant, so it drops entirely.  This removes the [S,B,N] projection
(274 GFLOP -> 0.4 GFLOP) and makes the kernel purely HBM-bandwidth-bound on a
single streaming pass over encoder_outputs.

Distribution: batch (B=64) data-parallel over 8 cores, 8 batch rows per core.
encoder_outputs/hidden are split on B, W is replicated; softmax is per-row so
no cross-device communication is needed.

Per-core dataflow (standard-ISA instructions only):
  - u = hT.T @ W on TensorE, bounced through DRAM and broadcast-DMA'd to all
    128 partitions (engines cannot cross partitions; DMA can).
  - Stream enc in [128, SBLK, 1024] tiles (s on partitions, n on free):
    VectorE tensor_mul by u_bc, then ScalarE activation(Identity,
    accum_out=...) performs the free-dim reduction -> scores[s_part, b, st].
  - Scores bounce through DRAM into natural [BPC, S] layout, then an exact
    per-row softmax with free-dim ops (reduce_max, Exp+accum, reciprocal,
    scale) and a contiguous output DMA.
"""

import os
import sys

import numpy as np

for _p in ("/root/.axon_site/_ro/trn_rl_repo", "/opt/trn_rl_repo"):
    if os.path.isdir(_p) and _p not in sys.path:
        sys.path.append(_p)

from contextlib import ExitStack

import concourse.bacc as bacc
import concourse.tile as tile
from concourse import mybir

F32 = mybir.dt.float32

S, B, N = 2048, 64, 1024
NCORES = 8
BPC = B // NCORES  # batches per core


def build(s=S, bpc=BPC, n=N, sblk=2, dma_engine="sync"):
    """Build the per-core Bass program (SPMD; identical on all cores)."""
    P = 128
    assert s % P == 0 and n % P == 0 and n % 512 == 0
    ST = s // P        # number of s-tiles (free-dim column per s-tile)
    KC = n // P        # contraction chunks for u = h @ W
    FB = n // 512      # psum free-dim blocks (fp32 moving max = 512)
    sblk = min(sblk, ST)
    assert ST % sblk == 0
    NBLK = ST // sblk

    # Bacc (not raw Bass): its compile pipeline fuses multi-sem waits into
    # event-semaphore instructions; raw Bass waits overflow walrus's per-
    # instruction sync-wait slots ("Too many sync wait commands").
    nc = bacc.Bacc("TRN2", target_bir_lowering=False, debug=False)
    enc = nc.declare_dram_parameter("enc", [s, bpc, n], F32, isOutput=False)
    hT = nc.declare_dram_parameter("hT", [n, bpc], F32, isOutput=False)
    w = nc.declare_dram_parameter("w", [n, n], F32, isOutput=False)
    out = nc.declare_dram_parameter("out", [bpc, s], F32, isOutput=True)

    dma = getattr(nc, dma_engine)

    with ExitStack() as ctx:
        tc = ctx.enter_context(tile.TileContext(nc))
        singles = ctx.enter_context(tc.tile_pool(name="singles", bufs=1))
        psum_pool = ctx.enter_context(tc.tile_pool(name="psum", bufs=1, space="PSUM"))
        psum_bc = ctx.enter_context(tc.tile_pool(name="psumbc", bufs=2, space="PSUM"))
        dramp = ctx.enter_context(tc.tile_pool(name="dram", bufs=1, space="DRAM"))

        # --- weights / hidden in SBUF ---
        # h_sb[p, c, b] = hidden[b, c*128 + p]
        h_sb = singles.tile([P, KC, bpc], F32)
        dma.dma_start(out=h_sb, in_=hT.rearrange("(c p) b -> p c b", p=P))
        # w_sb[p, c, n'] = W[c*128 + p, n']; one DMA per m-chunk so the u
        # matmuls for chunk c start as soon as chunk c lands.
        w_r = w.rearrange("(c p) n -> p c n", p=P)
        w_sb = singles.tile([P, KC, n], F32)
        for c in range(KC):
            dma.dma_start(out=w_sb[:, c, :], in_=w_r[:, c, :])
        ones_sb = singles.tile([1, P], F32)
        nc.vector.memset(ones_sb, 1.0)

        # --- b=0's u, broadcast to all partitions directly on TensorE:
        # stationary = h[0, m-chunk] replicated across all 128 M columns
        # (stride-0 free dim), moving = W chunk.  c-outer so the accumulation
        # completes right after the last W chunk lands -- this is the
        # first-STT critical path.  (~27us of PE, but PE is otherwise idle.)
        psum_ubc0 = psum_pool.tile([P, n], F32, tag="ubc0")
        for c in range(KC):
            for fb in range(FB):
                fsl = slice(fb * 512, (fb + 1) * 512)
                nc.tensor.matmul(
                    psum_ubc0[:, fsl],
                    lhsT=h_sb[:, c, 0:1].to_broadcast([P, P]),
                    rhs=w_sb[:, c, fsl],
                    start=(c == 0),
                    stop=(c == KC - 1),
                )

        # --- u[b, n'] for all b (M=8, cheap) for the remaining batches ---
        psum_u = psum_pool.tile([bpc, n], F32, tag="u")
        for c in range(KC):
            for fb in range(FB):
                fsl = slice(fb * 512, (fb + 1) * 512)
                nc.tensor.matmul(
                    psum_u[:, fsl],
                    lhsT=h_sb[:, c, :],
                    rhs=w_sb[:, c, fsl],
                    start=(c == 0),
                    stop=(c == KC - 1),
                )
        u_rows = singles.tile([bpc, n], F32)
        nc.scalar.copy(u_rows, psum_u)
        # relocate each u row to partition 0 (engines can't cross partitions;
        # tiny DMAs can).  SWDGE (gpsimd) keeps these off the enc-stream
        # HWDGE rings; DVE's shared SBUF port is free (in1 reads PSUM), so
        # SWDGE descriptor generation isn't blocked.
        u_r0 = singles.tile([1, bpc, n], F32)
        for bi in range(1, bpc):
            nc.gpsimd.dma_start(out=u_r0[0:1, bi, :], in_=u_rows[bi : bi + 1, :])

        encp = ctx.enter_context(tc.tile_pool(name="encp", bufs=12))
        scrp = ctx.enter_context(tc.tile_pool(name="scr", bufs=2))
        smp = ctx.enter_context(tc.tile_pool(name="smp", bufs=1))

        # s index mapping: s = p*ST + st (partition-major) so per-b scores
        # [128, ST] land contiguous when bounced to DRAM as [b, s].
        enc_r = enc.rearrange("(p st) b n -> p st b n", p=P)

        scores = singles.tile([P, bpc, ST], F32)
        scores_dram = dramp.tile([bpc, s], F32)
        scores_dram_r = scores_dram[:].rearrange("b (p st) -> p b st", p=P)

        for bi in range(bpc):
            if bi == 0:
                psum_ubc = psum_ubc0
            else:
                # u_bc[p, n'] = u[bi, n'] broadcast to all partitions via a
                # K=1 outer-product matmul: ones[1,128].T @ u_r0[0:1, fsl]
                # -> PSUM.  DVE reads in1 straight from PSUM (fp32 tensor
                # ops are 1x either way).
                psum_ubc = psum_bc.tile([P, n], F32, tag="ubc")
                for fb in range(FB):
                    fsl = slice(fb * 512, (fb + 1) * 512)
                    nc.tensor.matmul(
                        psum_ubc[:, fsl],
                        lhsT=ones_sb,
                        rhs=u_r0[0:1, bi, fsl],
                        start=True,
                        stop=True,
                    )

            for blk in range(NBLK):
                et = encp.tile([P, sblk, n], F32)
                # alternate the two HWDGE rings (SP / ACT) so consecutive
                # transfers overlap their completion latency
                eng = nc.scalar if (bi * NBLK + blk) % 2 == 0 else nc.sync
                eng.dma_start(
                    out=et, in_=enc_r[:, blk * sblk : (blk + 1) * sblk, bi, :]
                )
                for j in range(sblk):
                    st_i = blk * sblk + j
                    # fused multiply + free-dim reduce on DVE:
                    #   dump = (et + 0.0) * u_bc ; scores[...] = sum(dump)
                    dump = scrp.tile([P, n], F32, tag="dump")
                    nc.vector.scalar_tensor_tensor(
                        out=dump,
                        in0=et[:, j, :],
                        scalar=0.0,
                        in1=psum_ubc,
                        op0=mybir.AluOpType.add,
                        op1=mybir.AluOpType.mult,
                        accum_out=scores[:, bi, st_i : st_i + 1],
                    )
            # bounce this b's scores to DRAM in natural [b, s] layout
            # (SWDGE: keep the HWDGE rings pure for the enc stream)
            nc.gpsimd.dma_start(out=scores_dram_r[:, bi, :], in_=scores[:, bi, :])

        # --- softmax over s, rows natural in [bpc, s]; two halves so the
        # first half overlaps the second half's streaming ---
        half = max(1, bpc // 2)
        for h0 in range(0, bpc, half):
            hsl = slice(h0, h0 + half)
            hn = min(half, bpc - h0)
            sc = smp.tile([hn, s], F32, tag="sc")
            nc.gpsimd.dma_start(out=sc, in_=scores_dram[hsl, :])
            m = smp.tile([hn, 1], F32, tag="m")
            nc.vector.reduce_max(out=m, in_=sc, axis=mybir.AxisListType.X)
            negm = smp.tile([hn, 1], F32, tag="negm")
            nc.vector.tensor_scalar_mul(negm, m, -1.0)
            ssum = smp.tile([hn, 1], F32, tag="ssum")
            nc.scalar.activation(
                out=sc,
                in_=sc,
                func=mybir.ActivationFunctionType.Exp,
                bias=negm,
                scale=1.0,
                accum_out=ssum,
            )
            inv = smp.tile([hn, 1], F32, tag="inv")
            nc.vector.reciprocal(inv, ssum)
            nc.vector.tensor_scalar_mul(sc, sc, inv)
            nc.gpsimd.dma_start(out=out[hsl, :], in_=sc)

    nc.finalize()
    return nc


def make_in_maps(hidden, encoder_outputs, W):
    hT_all = np.ascontiguousarray(hidden[0].T)  # [N, B]
    in_maps = []
    for c in range(NCORES):
        bsl = slice(c * BPC, (c + 1) * BPC)
        in_maps.append(
            {
                "enc": np.ascontiguousarray(encoder_outputs[:, bsl, :]),
                "hT": np.ascontiguousarray(hT_all[:, bsl]),
                "w": np.ascontiguousarray(W),
            }
        )
    return in_maps


def _install_ntff_shim():
    """The agent image's antenv package lacks axon_hooks; recreate it so
    trace=True can capture NTFF profiles. Harness runs never use this."""
    import types

    name = "antenv.axon_hooks"
    if name in sys.modules:
        return
    try:
        mod = types.ModuleType(name)
        mod._hook = None
        mod.set_axon_ntff_profile_hook = lambda h: setattr(mod, "_hook", h)
        mod.get_axon_ntff_profile_hook = lambda: mod._hook
        sys.modules[name] = mod
        if "/root/.axon_site" not in sys.path:
            sys.path.insert(0, "/root/.axon_site")
        from trn_agent_boot.trn_boot import _ntff_profile_via_ctypes

        mod._hook = _ntff_profile_via_ctypes("/opt/axon/libaxon_pjrt.so")
    except Exception:
        pass


def kernel(hidden, encoder_outputs, W, b, _trace=False):
    """Full-input entry point. `b` (bias) is mathematically irrelevant
    (softmax shift invariance) and unused."""
    if _trace:
        _install_ntff_shim()
    from concourse.bass_utils import run_bass_kernel_spmd

    hidden = np.asarray(hidden, dtype=np.float32)
    encoder_outputs = np.asarray(encoder_outputs, dtype=np.float32)
    W = np.asarray(W, dtype=np.float32)

    nc = build()
    in_maps = make_in_maps(hidden, encoder_outputs, W)
    res = run_bass_kernel_spmd(nc, in_maps, list(range(NCORES)), trace=_trace)
    full = np.concatenate([r["out"] for r in res.results], axis=0)  # [B, S]
    out = full[:, None, :].astype(np.float32)
    if _trace:
        return out, res
    return out

